# revision 10
# baseline (speedup 1.0000x reference)
"""Trainium2 Bass kernel for BiLinearAttention.

scores[b,d,e] = enc[b,e] @ W @ dec[b,d] + bias

Full inputs (as produced by setup_inputs):
  encoder_output [8, 2048, 512] f32
  decoder_output [8, 2048, 512] f32
  W              [1, 512, 512]  f32
  b              [1]            f32
Output: scores [8, 2048, 2048] f32

Sharding: batch across the 8 cores (data parallel, no collectives).

Per-core plan (S=2048, H=512, P=128):
  1. PE-transpose enc tiles -> encT[h, e]   (feature-major)
  2. encWT[g, e] = sum_h W[h,g] * encT[h,e]   (lhsT = W as stored, fp32r)
  3. PE-transpose dec tiles -> decT[g, d]
  4. scores[d, e] = sum_g decT[g,d] * encWT[g,e]  (fp32r matmuls)
  5. evict PSUM + bias on DVE, DMA out

fp32r (TF32-like, ~1e-4 rel err) runs the PE at 1 cycle/row vs 4 for fp32.
Emission is software-pipelined so the PE never waits on ACT evictions:
transposes of chunk c+1 are interleaved ahead of the matmuls that consume
chunk c. A short burst of dummy matmuls at t=0 warms the PE clock (HAM)
while the first DMAs land.
"""

import numpy as np

import concourse.bass as bass
import concourse.tile as tile
from concourse import bacc, mybir
from concourse.bass_utils import run_bass_kernel_spmd
from concourse.masks import make_identity

dt = mybir.dt

B = 8
S = 2048          # seq len (enc and dec)
H = 512           # hidden
P = 128           # partitions
NKT = H // P      # 4 k-tiles over the feature dim
NC_CHUNK = 512    # matmul moving free dim / seq chunk
NCH = S // NC_CHUNK  # 4 seq chunks
NJT = NC_CHUNK // P  # 4 seq tiles per chunk

f32 = dt.float32
f32r = dt.float32r


def build_program():
    nc = bacc.Bacc("TRN2", target_bir_lowering=False, debug=False, num_devices=B)
    enc = nc.dram_tensor("enc", [S, H], f32, kind="ExternalInput").ap()
    dec = nc.dram_tensor("dec", [S, H], f32, kind="ExternalInput").ap()
    w = nc.dram_tensor("w", [H, H], f32, kind="ExternalInput").ap()
    bias = nc.dram_tensor("bias", [1], f32, kind="ExternalInput").ap()
    out = nc.dram_tensor("out", [S, S], f32, kind="ExternalOutput").ap()

    # DRAM views: [seq, h] -> [chunk, jt, p, h] with seq = c*512 + jt*128 + p
    enc_vc = enc.rearrange("(c j p) h -> c p j h", c=NCH, j=NJT, p=P)
    dec_vc = dec.rearrange("(c j p) h -> c p j h", c=NCH, j=NJT, p=P)
    w_v = w.rearrange("(kt p) g -> p kt g", p=P)

    with tile.TileContext(nc) as tc, tc.tile_pool(name="singles", bufs=1) as sg:
        # PE warmup tiles (no DMA deps). fp32 matmuls run 4 cycles/row,
        # so 2 of them give the ~3.4us of PE activity that flips HAM to
        # full clock before the real work arrives.
        warm_l32 = sg.tile([P, P], f32)
        nc.vector.memset(warm_l32, 0.0)
        warm_r32 = sg.tile([P, NC_CHUNK], f32)
        nc.vector.memset(warm_r32, 0.0)

        # constants
        ident32 = sg.tile([P, P], f32)
        make_identity(nc, ident32)
        ident = sg.tile([P, P], f32r)
        nc.scalar.copy(ident, ident32)

        w_sb = sg.tile([P, NKT, H], f32r)

        # bias -> [128, 1] column via K=1 matmul against ones
        ones_row = sg.tile([1, P], f32)
        nc.vector.memset(ones_row, 1.0)
        b_sb1 = sg.tile([1, 1], f32)
        nc.sync.dma_start(b_sb1, bias)
        bias_col = sg.tile([P, 1], f32)

        # feature-major persistents, chunked over seq for fine-grained deps
        encT = [sg.tile([P, NKT, NC_CHUNK], f32r, name=f"encT{c}") for c in range(NCH)]
        decT = [sg.tile([P, NKT, NC_CHUNK], f32r, name=f"decT{c}") for c in range(NCH)]
        encWT = [sg.tile([P, NKT, NC_CHUNK], f32r, name=f"encWT{c}") for c in range(NCH)]

        with (
            tc.tile_pool(name="raw_e", bufs=3) as rep,
            tc.tile_pool(name="raw_d", bufs=4) as rdp,
            tc.tile_pool(name="ps", bufs=8, space="PSUM") as psp,
            tc.tile_pool(name="ob", bufs=3) as obp,
        ):
            # ---- PE warmup: engage HAM K=8/8 while first DMAs land ----
            ps_w = psp.tile([P, NC_CHUNK], f32, tag="ps", name="ps_w")
            for _ in range(2):
                nc.tensor.matmul(ps_w, warm_l32, warm_r32, start=True, stop=True)

            def pe_filler():
                nc.tensor.matmul(ps_w, warm_l32, warm_r32, start=True, stop=True)

            # ---- input DMAs: chunk-level (dma_start costs ~0.7us of
            # sequencer time each, so batch big). First enc chunk is split
            # in half so the first transposes start ~1.5us earlier.
            raw_e = []
            raw_d = []
            e0a = rep.tile([P, 2, H], f32r, tag="re2", name="raw_e0a")
            nc.sync.dma_start(e0a, enc_vc[0][:, 0:2].bitcast(f32r))
            e0b = rep.tile([P, 2, H], f32r, tag="re2", name="raw_e0b")
            nc.sync.dma_start(e0b, enc_vc[0][:, 2:4].bitcast(f32r))
            raw_e.append((e0a, e0b))
            nc.sync.dma_start(w_sb[:, :, 0:H // 2],
                              w_v[:, :, 0:H // 2].bitcast(f32r))
            nc.sync.dma_start(w_sb[:, :, H // 2:],
                              w_v[:, :, H // 2:].bitcast(f32r))
            for c in range(1, NCH):
                te = rep.tile([P, NJT, H], f32r, tag="re", name=f"raw_e{c}")
                nc.sync.dma_start(te, enc_vc[c].bitcast(f32r))
                raw_e.append(te)
            for c in range(NCH):
                td = rdp.tile([P, NJT, H], f32r, tag="rd", name=f"raw_d{c}")
                nc.sync.dma_start(td, dec_vc[c].bitcast(f32r))
                raw_d.append(td)

            def tp_chunk(raw_c, dst, js=range(NJT)):
                # raw tiles [128(seq), H] -> dst [128(feat), kt, 512(seq)]
                for j in js:
                    if isinstance(raw_c, tuple):
                        src_t = raw_c[j // 2][:, j % 2]
                    else:
                        src_t = raw_c[:, j]
                    tp = psp.tile([P, NKT, P], f32r, tag="ps", name="tp")
                    for kt in range(NKT):
                        nc.tensor.transpose(
                            tp[:, kt], src_t[:, kt * P:(kt + 1) * P], ident
                        )
                    dst_sl = dst[:, :, j * P:(j + 1) * P]
                    if j % 2 == 0:
                        nc.scalar.copy(dst_sl, tp)
                    else:
                        nc.vector.tensor_copy(dst_sl, tp)

            def step1(c, mid_hook=None):
                # encWT[c][g, e] = sum_h W[h,g] encT[c][h,e]
                for gt in range(NKT):
                    if gt == 2 and mid_hook is not None:
                        mid_hook()
                    ps1 = psp.tile([P, NC_CHUNK], f32, tag="ps", name="ps1")
                    for kt in range(NKT):
                        nc.tensor.matmul(
                            ps1,
                            w_sb[:, kt, gt * P:(gt + 1) * P],
                            encT[c][:, kt],
                            start=(kt == 0),
                            stop=(kt == NKT - 1),
                        )
                    nc.scalar.copy(encWT[c][:, gt], ps1)

            def evict(dst_sl, ps, ec):
                if ec % 2 == 0:
                    nc.vector.tensor_scalar(
                        dst_sl, ps, scalar1=bias_col, scalar2=None,
                        op0=mybir.AluOpType.add,
                    )
                else:
                    nc.scalar.activation(
                        dst_sl, ps,
                        mybir.ActivationFunctionType.Identity, bias=bias_col,
                    )

            def step2_dtile(c, jd):
                dt_row = c * NJT + jd  # output row-block index
                last = dt_row == S // P - 1
                out_rows = out[dt_row * P:(dt_row + 1) * P, :]
                ps2 = [
                    psp.tile([P, NC_CHUNK], f32, tag="ps", name=f"ps2_{ec}")
                    for ec in range(NCH)
                ]
                if not last:
                    for gt in range(NKT):
                        lhsT = decT[c][:, gt, jd * P:(jd + 1) * P]
                        for ec in range(NCH):
                            nc.tensor.matmul(
                                ps2[ec],
                                lhsT,
                                encWT[ec][:, gt],
                                start=(gt == 0),
                                stop=(gt == NKT - 1),
                            )
                    ob = obp.tile([P, S], f32, tag="ob", name="ob")
                    for ec in range(NCH):
                        evict(ob[:, ec * NC_CHUNK:(ec + 1) * NC_CHUNK],
                              ps2[ec], ec)
                    if dt_row % 2 == 0:
                        nc.sync.dma_start(out_rows, ob)
                    else:
                        nc.scalar.dma_start(out_rows, ob)
                else:
                    # final row-block: ec-major so eviction + store of each
                    # 512-slice overlaps the remaining matmuls (short tail)
                    for ec in range(NCH):
                        for gt in range(NKT):
                            nc.tensor.matmul(
                                ps2[ec],
                                decT[c][:, gt, jd * P:(jd + 1) * P],
                                encWT[ec][:, gt],
                                start=(gt == 0),
                                stop=(gt == NKT - 1),
                            )
                        obs = obp.tile([P, NC_CHUNK], f32, tag="obs", name="obs")
                        evict(obs, ps2[ec], ec)
                        eng = nc.sync if ec % 2 == 0 else nc.scalar
                        eng.dma_start(
                            out_rows[:, ec * NC_CHUNK:(ec + 1) * NC_CHUNK], obs
                        )

            # ---- software-pipelined emission ----
            # encoder: transpose + step1 per chunk; next chunk's transposes
            # are emitted mid-step1 so encT evictions overlap matmuls.
            # pe_filler() keeps the PE HAM-warm across DMA-paced waits.
            tp_chunk(raw_e[0], encT[0])
            pe_filler()
            for c in range(NCH):
                hook = None
                if c + 1 < NCH:
                    nxt = c + 1
                    def hook(n=nxt):
                        pe_filler()
                        tp_chunk(raw_e[n], encT[n])
                        pe_filler()
                step1(c, mid_hook=hook)
            # bias column (needed by the first step2 eviction)
            ps_b = psp.tile([P, 1], f32, tag="ps", name="ps_b")
            nc.tensor.matmul(ps_b, ones_row, b_sb1, start=True, stop=True)
            nc.vector.tensor_copy(bias_col, ps_b)
            # decoder: transposes run one chunk ahead of their consumers
            pe_filler()
            tp_chunk(raw_d[0], decT[0])
            pe_filler()
            tp_chunk(raw_d[1], decT[1])
            for c in range(NCH):
                for jd in range(NJT):
                    step2_dtile(c, jd)
                    tc_next = c + 2
                    if tc_next < NCH and jd < 2:
                        tp_chunk(raw_d[tc_next], decT[tc_next],
                                 js=range(2 * jd, 2 * jd + 2))

    nc.compile()
    return nc


_NC = None


def _get_nc():
    global _NC
    if _NC is None:
        _NC = build_program()
    return _NC


def run(inputs, trace=False, **kw):
    nc = _get_nc()
    enc = np.ascontiguousarray(inputs["encoder_output"], dtype=np.float32)
    dec = np.ascontiguousarray(inputs["decoder_output"], dtype=np.float32)
    w = np.ascontiguousarray(inputs["W"][0], dtype=np.float32)
    b = np.ascontiguousarray(inputs["b"], dtype=np.float32)
    in_maps = [
        {"enc": enc[i], "dec": dec[i], "w": w, "bias": b} for i in range(B)
    ]
    res = run_bass_kernel_spmd(nc, in_maps, list(range(B)), trace=trace, **kw)
    out = np.stack([res.results[i]["out"] for i in range(B)], axis=0)
    return out, res


def kernel(**inputs) -> np.ndarray:
    out, _ = run(inputs)
    return out


# revision 11
# speedup vs baseline: 1.0386x; 1.0386x over previous
"""Trainium2 Bass kernel for BiLinearAttention.

scores[b,d,e] = enc[b,e] @ W @ dec[b,d] + bias

Full inputs (as produced by setup_inputs):
  encoder_output [8, 2048, 512] f32
  decoder_output [8, 2048, 512] f32
  W              [1, 512, 512]  f32
  b              [1]            f32
Output: scores [8, 2048, 2048] f32

Sharding: batch across the 8 cores (data parallel, no collectives).

Per-core plan (S=2048, H=512, P=128):
  1. PE-transpose enc tiles -> encT[h, e]   (feature-major)
  2. encWT[g, e] = sum_h W[h,g] * encT[h,e]   (lhsT = W as stored, fp32r)
  3. PE-transpose dec tiles -> decT[g, d]
  4. scores[d, e] = sum_g decT[g,d] * encWT[g,e]  (fp32r matmuls)
  5. evict PSUM + bias on DVE, DMA out

fp32r (TF32-like, ~1e-4 rel err) runs the PE at 1 cycle/row vs 4 for fp32.
Emission is software-pipelined so the PE never waits on ACT evictions:
transposes of chunk c+1 are interleaved ahead of the matmuls that consume
chunk c. A short burst of dummy matmuls at t=0 warms the PE clock (HAM)
while the first DMAs land.
"""

import numpy as np

import concourse.bass as bass
import concourse.tile as tile
from concourse import bacc, mybir
from concourse.bass_utils import run_bass_kernel_spmd
from concourse.masks import make_identity

dt = mybir.dt

B = 8
S = 2048          # seq len (enc and dec)
H = 512           # hidden
P = 128           # partitions
NKT = H // P      # 4 k-tiles over the feature dim
NC_CHUNK = 512    # matmul moving free dim / seq chunk
NCH = S // NC_CHUNK  # 4 seq chunks
NJT = NC_CHUNK // P  # 4 seq tiles per chunk

f32 = dt.float32
f32r = dt.float32r


def build_program():
    nc = bacc.Bacc("TRN2", target_bir_lowering=False, debug=False, num_devices=B)
    enc = nc.dram_tensor("enc", [S, H], f32, kind="ExternalInput").ap()
    dec = nc.dram_tensor("dec", [S, H], f32, kind="ExternalInput").ap()
    w = nc.dram_tensor("w", [H, H], f32, kind="ExternalInput").ap()
    bias = nc.dram_tensor("bias", [1], f32, kind="ExternalInput").ap()
    out = nc.dram_tensor("out", [S, S], f32, kind="ExternalOutput").ap()

    # DRAM views: [seq, h] -> [chunk, jt, p, h] with seq = c*512 + jt*128 + p
    enc_vc = enc.rearrange("(c j p) h -> c p j h", c=NCH, j=NJT, p=P)
    dec_vc = dec.rearrange("(c j p) h -> c p j h", c=NCH, j=NJT, p=P)
    w_v = w.rearrange("(kt p) g -> p kt g", p=P)

    with tile.TileContext(nc) as tc, tc.tile_pool(name="singles", bufs=1) as sg:
        # PE warmup tiles (no DMA deps). fp32 matmuls run 4 cycles/row,
        # so 2 of them give the ~3.4us of PE activity that flips HAM to
        # full clock before the real work arrives.
        warm_l32 = sg.tile([P, P], f32)
        nc.vector.memset(warm_l32, 0.0)
        warm_r32 = sg.tile([P, NC_CHUNK], f32)
        nc.vector.memset(warm_r32, 0.0)

        # constants
        ident32 = sg.tile([P, P], f32)
        make_identity(nc, ident32)
        ident = sg.tile([P, P], f32r)
        nc.scalar.copy(ident, ident32)

        w_sb = sg.tile([P, NKT, H], f32r)

        # bias -> [128, 1] column via K=1 matmul against ones
        ones_row = sg.tile([1, P], f32)
        nc.vector.memset(ones_row, 1.0)
        b_sb1 = sg.tile([1, 1], f32)
        bias_col = sg.tile([P, 1], f32)

        # feature-major persistents, chunked over seq for fine-grained deps
        encT = [sg.tile([P, NKT, NC_CHUNK], f32r, name=f"encT{c}") for c in range(NCH)]
        decT = [sg.tile([P, NKT, NC_CHUNK], f32r, name=f"decT{c}") for c in range(NCH)]
        encWT = [sg.tile([P, NKT, NC_CHUNK], f32r, name=f"encWT{c}") for c in range(NCH)]

        with (
            tc.tile_pool(name="raw_e", bufs=3) as rep,
            tc.tile_pool(name="raw_d", bufs=4) as rdp,
            tc.tile_pool(name="ps", bufs=8, space="PSUM") as psp,
            tc.tile_pool(name="ob", bufs=3) as obp,
        ):
            # ---- PE warmup: engage HAM K=8/8 while first DMAs land ----
            ps_w = psp.tile([P, NC_CHUNK], f32, tag="ps", name="ps_w")
            for _ in range(2):
                nc.tensor.matmul(ps_w, warm_l32, warm_r32, start=True, stop=True)

            def pe_filler():
                # tiny fp32 matmul (~213ns): registers PE activity so the
                # HAM idle-window never re-throttles during DMA waits
                nc.tensor.matmul(ps_w[:, 0:P], warm_l32, warm_r32[:, 0:P],
                                 start=True, stop=True)

            # ---- input DMAs: chunk-level (dma_start costs ~0.7us of
            # sequencer time each, so batch big). First enc chunk is split
            # in half so the first transposes start ~1.5us earlier.
            raw_e = []
            raw_d = []
            e0a = rep.tile([P, 2, H], f32r, tag="re2", name="raw_e0a")
            nc.sync.dma_start(e0a, enc_vc[0][:, 0:2].bitcast(f32r))
            e0b = rep.tile([P, 2, H], f32r, tag="re2", name="raw_e0b")
            nc.sync.dma_start(e0b, enc_vc[0][:, 2:4].bitcast(f32r))
            raw_e.append((e0a, e0b))
            nc.sync.dma_start(w_sb[:, :, 0:H // 2],
                              w_v[:, :, 0:H // 2].bitcast(f32r))
            nc.sync.dma_start(w_sb[:, :, H // 2:],
                              w_v[:, :, H // 2:].bitcast(f32r))
            nc.sync.dma_start(b_sb1, bias)
            for c in range(1, NCH):
                te = rep.tile([P, NJT, H], f32r, tag="re", name=f"raw_e{c}")
                nc.sync.dma_start(te, enc_vc[c].bitcast(f32r))
                raw_e.append(te)
            for c in range(NCH):
                td = rdp.tile([P, NJT, H], f32r, tag="rd", name=f"raw_d{c}")
                nc.sync.dma_start(td, dec_vc[c].bitcast(f32r))
                raw_d.append(td)

            def tp_chunk(raw_c, dst, js=range(NJT)):
                # raw tiles [128(seq), H] -> dst [128(feat), kt, 512(seq)]
                for j in js:
                    if isinstance(raw_c, tuple):
                        src_t = raw_c[j // 2][:, j % 2]
                    else:
                        src_t = raw_c[:, j]
                    tp = psp.tile([P, NKT, P], f32r, tag="ps", name="tp")
                    for kt in range(NKT):
                        nc.tensor.transpose(
                            tp[:, kt], src_t[:, kt * P:(kt + 1) * P], ident
                        )
                    dst_sl = dst[:, :, j * P:(j + 1) * P]
                    if j % 2 == 0:
                        nc.scalar.copy(dst_sl, tp)
                    else:
                        nc.vector.tensor_copy(dst_sl, tp)

            def step1(c, mid_hook=None):
                # encWT[c][g, e] = sum_h W[h,g] encT[c][h,e]
                for gt in range(NKT):
                    if gt == 2 and mid_hook is not None:
                        mid_hook()
                    ps1 = psp.tile([P, NC_CHUNK], f32, tag="ps", name="ps1")
                    for kt in range(NKT):
                        nc.tensor.matmul(
                            ps1,
                            w_sb[:, kt, gt * P:(gt + 1) * P],
                            encT[c][:, kt],
                            start=(kt == 0),
                            stop=(kt == NKT - 1),
                        )
                    nc.scalar.copy(encWT[c][:, gt], ps1)

            def evict(dst_sl, ps, ec):
                if ec % 2 == 0:
                    nc.vector.tensor_scalar(
                        dst_sl, ps, scalar1=bias_col, scalar2=None,
                        op0=mybir.AluOpType.add,
                    )
                else:
                    nc.scalar.activation(
                        dst_sl, ps,
                        mybir.ActivationFunctionType.Identity, bias=bias_col,
                    )

            def step2_dtile(c, jd):
                dt_row = c * NJT + jd  # output row-block index
                last = dt_row == S // P - 1
                out_rows = out[dt_row * P:(dt_row + 1) * P, :]
                ps2 = [
                    psp.tile([P, NC_CHUNK], f32, tag="ps", name=f"ps2_{ec}")
                    for ec in range(NCH)
                ]
                if not last:
                    for gt in range(NKT):
                        lhsT = decT[c][:, gt, jd * P:(jd + 1) * P]
                        for ec in range(NCH):
                            nc.tensor.matmul(
                                ps2[ec],
                                lhsT,
                                encWT[ec][:, gt],
                                start=(gt == 0),
                                stop=(gt == NKT - 1),
                            )
                    ob = obp.tile([P, S], f32, tag="ob", name="ob")
                    for ec in range(NCH):
                        evict(ob[:, ec * NC_CHUNK:(ec + 1) * NC_CHUNK],
                              ps2[ec], ec)
                    if dt_row % 2 == 0:
                        nc.sync.dma_start(out_rows, ob)
                    else:
                        nc.scalar.dma_start(out_rows, ob)
                else:
                    # final row-block: ec-major so eviction + store of each
                    # 512-slice overlaps the remaining matmuls (short tail)
                    for ec in range(NCH):
                        for gt in range(NKT):
                            nc.tensor.matmul(
                                ps2[ec],
                                decT[c][:, gt, jd * P:(jd + 1) * P],
                                encWT[ec][:, gt],
                                start=(gt == 0),
                                stop=(gt == NKT - 1),
                            )
                        obs = obp.tile([P, NC_CHUNK], f32, tag="obs", name="obs")
                        evict(obs, ps2[ec], ec)
                        eng = nc.sync if ec % 2 == 0 else nc.scalar
                        eng.dma_start(
                            out_rows[:, ec * NC_CHUNK:(ec + 1) * NC_CHUNK], obs
                        )

            # ---- software-pipelined emission ----
            # encoder: transpose + step1 per chunk; next chunk's transposes
            # are emitted mid-step1 so encT evictions overlap matmuls.
            # pe_filler() keeps the PE HAM-warm across DMA-paced waits.
            tp_chunk(raw_e[0], encT[0])
            pe_filler()
            for c in range(NCH):
                hook = None
                if c + 1 < NCH:
                    nxt = c + 1
                    def hook(n=nxt):
                        pe_filler()
                        tp_chunk(raw_e[n], encT[n])
                        pe_filler()
                step1(c, mid_hook=hook)
            # bias column (needed by the first step2 eviction)
            ps_b = psp.tile([P, 1], f32, tag="ps", name="ps_b")
            nc.tensor.matmul(ps_b, ones_row, b_sb1, start=True, stop=True)
            nc.vector.tensor_copy(bias_col, ps_b)
            # decoder: transposes run one chunk ahead of their consumers
            pe_filler()
            tp_chunk(raw_d[0], decT[0])
            pe_filler()
            tp_chunk(raw_d[1], decT[1])
            for c in range(NCH):
                for jd in range(NJT):
                    step2_dtile(c, jd)
                    tc_next = c + 2
                    if tc_next < NCH and jd < 2:
                        tp_chunk(raw_d[tc_next], decT[tc_next],
                                 js=range(2 * jd, 2 * jd + 2))

    nc.compile()
    return nc


_NC = None


def _get_nc():
    global _NC
    if _NC is None:
        _NC = build_program()
    return _NC


def run(inputs, trace=False, **kw):
    nc = _get_nc()
    enc = np.ascontiguousarray(inputs["encoder_output"], dtype=np.float32)
    dec = np.ascontiguousarray(inputs["decoder_output"], dtype=np.float32)
    w = np.ascontiguousarray(inputs["W"][0], dtype=np.float32)
    b = np.ascontiguousarray(inputs["b"], dtype=np.float32)
    in_maps = [
        {"enc": enc[i], "dec": dec[i], "w": w, "bias": b} for i in range(B)
    ]
    res = run_bass_kernel_spmd(nc, in_maps, list(range(B)), trace=trace, **kw)
    out = np.stack([res.results[i]["out"] for i in range(B)], axis=0)
    return out, res


def kernel(**inputs) -> np.ndarray:
    out, _ = run(inputs)
    return out
